# revision 10
# baseline (speedup 1.0000x reference)
"""Trainium2 Bass kernel for AttentionPooling (B=4, S=8192, Q=1024, H=768).

Sharding: data-parallel over (batch, query-half) -> 8 shards, one per core,
no collectives. Per core:
  X   = hidden_states[b]            [8192, 768]
  Qin = queries[b, qh*512:(qh+1)*512] [512, 768]

Algebra (per core):
  Qp^T = Wq^T(lhsT=Wq) @ Qin^T            [768d, 512q]   (+bq per-partition)
  WQ   = Wk @ Qp^T                        [768h, 512q]   (bk drops: softmax
                                                          is shift-invariant
                                                          along s)
  scores^T = X @ WQ  (lhsT = X^T tiles)   [8192s, 512q]
  E    = exp(scores^T/sqrt(H) + maskbias) (no max-subtraction: scores~N(0,1))
  T'   = E^T @ [X | 1]                    [512q, 769]    (col 768 = softmax
                                                          denominators)
  ctx  = (T'^T as lhsT) @ [Wv ; bv] * (1/sums)  [512q, 768]

All matmuls in bf16 (PSUM accumulation fp32); X/weights are cast fp32->bf16
during DMA (SWDGE). X^T is produced by XBAR DMA-transpose via a DRAM bounce.
"""

import math
import sys

import numpy as np

for _p in ("/opt/trn_rl_repo",):
    if _p not in sys.path:
        sys.path.insert(0, _p)

import concourse.bass as bass  # noqa: E402
import concourse.mybir as mybir  # noqa: E402
import concourse.tile as tile  # noqa: E402
from concourse import bacc, bass_utils  # noqa: E402
from concourse.masks import make_identity  # noqa: E402

B, S, QF, H = 4, 8192, 1024, 768
NCORES = 8
QL = QF // 2  # queries per core
P = 128
HT = H // P  # 6 h-tiles
SC = 1024  # s-chunk
NCH = S // SC  # 4 chunks
ST = SC // P  # 16 s-tiles per chunk
QT = QL // P  # 4 q-tiles
BF16 = mybir.dt.bfloat16
F32 = mybir.dt.float32
SCALE = 1.0 / math.sqrt(H)
AF = mybir.ActivationFunctionType

_cached_nc = None
LAST_RUN = None


def _install_ntff_hook():
    """Provide antenv.axon_hooks (absent in this image) so that
    run_bass_kernel_spmd(trace=True) / BASS_TRACE=1 can neuron-profile
    via the axon .so. Harmless no-op if already present."""
    import types

    try:
        import antenv.axon_hooks  # noqa: F401

        return
    except ImportError:
        pass
    hook = None
    try:
        from trn_agent_boot.trn_boot import _ntff_profile_via_ctypes

        hook = _ntff_profile_via_ctypes("/opt/axon/libaxon_pjrt.so")
    except Exception:
        hook = None
    try:
        import antenv

        mod = types.ModuleType("antenv.axon_hooks")
        mod._hook = hook
        mod.get_axon_ntff_profile_hook = lambda: mod._hook

        def _set(h):
            mod._hook = h

        mod.set_axon_ntff_profile_hook = _set
        sys.modules["antenv.axon_hooks"] = mod
        antenv.axon_hooks = mod
    except Exception:
        pass

    # upload_artifacts needs a bucket; in this sandbox make it best-effort
    # so a failed upload doesn't kill the profile pipeline.
    _orig_upload = bass_utils.upload_artifacts

    def _safe_upload(tmpdir):
        try:
            return _orig_upload(tmpdir)
        except Exception:
            return tmpdir

    bass_utils.upload_artifacts = _safe_upload


_install_ntff_hook()


def _body(nc, tc, hs, qin, msk, wq, wk, wv, bq, bv, out):
    from contextlib import ExitStack

    F32R = mybir.dt.float32r

    def r(ap):
        return ap.bitcast(F32R)

    with ExitStack() as ctx:
        const = ctx.enter_context(tc.tile_pool(name="const", bufs=1))
        wpool = ctx.enter_context(tc.tile_pool(name="weights", bufs=1))
        dram = ctx.enter_context(tc.tile_pool(name="dram", bufs=4, space="DRAM"))
        ps_s = ctx.enter_context(tc.tile_pool(name="ps_s", bufs=3, space="PSUM"))
        ps_t = ctx.enter_context(tc.tile_pool(name="ps_t", bufs=3, space="PSUM"))

        # ---------- constants ----------
        ident = const.tile([P, P], F32)
        make_identity(nc, ident[:])

        # persistent SBUF
        wqq = wpool.tile([P, HT, QL], BF16)  # WQ = Wk @ Qp^T [h, q]
        mb = const.tile([P, S // P], F32)  # mask bias per s (partition = s%128)
        bq_sb = const.tile([P, HT], F32)
        T_sb = wpool.tile([P, QT, H + 1], F32)  # T' accumulator [q, 769]
        wvF = wpool.tile([P, HT, H], F32)  # Wv natural [h, d] fp32
        bvF = const.tile([1, H], F32)

        # chunk schedule: two small chunks first to fill the pipeline fast
        chunk_sizes = [512, 512] + [1024] * 7
        assert sum(chunk_sizes) == S

        with (
            tc.tile_pool(name="pro", bufs=1) as pro,
            tc.tile_pool(name="ps_pro", bufs=2, space="PSUM") as ps_pro,
        ):
            # fp32 loads on the HWDGE queue, in critical-path order
            qf = pro.tile([P, QT, H], F32)
            nc.sync.dma_start(qf[:], qin.rearrange("(t p) h -> p t h", p=P))
            wkF = pro.tile([P, HT, H], F32)
            nc.sync.dma_start(wkF[:], wk.rearrange("(t p) d -> p t d", p=P))
            wqF = pro.tile([P, HT, H], F32)
            nc.sync.dma_start(wqF[:], wq.rearrange("(t p) d -> p t d", p=P))

            # mask / bq: load transposed (contiguous rows), PE-transpose back
            mk64 = pro.tile([S // P, P], F32)
            nc.sync.dma_start(mk64[:], msk.rearrange("(t p) -> t p", p=P))
            bq6 = pro.tile([HT, P], F32)
            nc.sync.dma_start(bq6[:], bq.rearrange("(t p) -> t p", p=P))

            # Qin^T via PE transpose (fp32)
            qinT = pro.tile([P, HT, QL], BF16)  # Qin^T [h, q]
            for ht in range(HT):
                psq = ps_pro.tile([P, QL], F32, tag="ps_pro", name="psq")
                for qt in range(QT):
                    nc.tensor.transpose(
                        psq[:, qt * P : (qt + 1) * P],
                        qf[:, qt, ht * P : (ht + 1) * P],
                        ident[:],
                    )
                nc.scalar.copy(qinT[:, ht, :], psq[:])

            # Wk^T via PE transpose (fp32)
            wkT = pro.tile([P, HT, H], BF16)  # Wk^T [d, h]
            for dt in range(HT):
                for g0, gn in ((0, 4), (4, 2)):
                    psw = ps_pro.tile([P, QL], F32, tag="ps_pro", name="psw")
                    for ht in range(g0, g0 + gn):
                        nc.tensor.transpose(
                            psw[:, (ht - g0) * P : (ht - g0 + 1) * P],
                            wkF[:, ht, dt * P : (dt + 1) * P],
                            ident[:],
                        )
                    nc.scalar.copy(
                        wkT[:, dt, g0 * P : (g0 + gn) * P], psw[:, : gn * P]
                    )

            # mask bias: transpose [64,128] -> [128,64], then (m-1)*1e4
            psm = ps_pro.tile([P, QL], F32, tag="ps_pro", name="psm")
            nc.tensor.transpose(psm[:, : S // P], mk64[:], ident[: S // P, : S // P])
            nc.vector.tensor_scalar(
                mb[:], psm[:, : S // P], 1.0, 10000.0,
                mybir.AluOpType.subtract, mybir.AluOpType.mult,
            )
            psb = ps_pro.tile([P, QL], F32, tag="ps_pro", name="psb")
            nc.tensor.transpose(psb[:, :HT], bq6[:], ident[:HT, :HT])
            nc.vector.tensor_copy(bq_sb[:], psb[:, :HT])

            wq_sb = pro.tile([P, HT, H], BF16)
            nc.scalar.copy(wq_sb[:], wqF[:])

            # Qp^T[d, q] = sum_h Wq[h, d] * Qin^T[h, q]   (+bq on partitions)
            qpT = pro.tile([P, HT, QL], BF16)
            for dt in range(HT):
                ps = ps_pro.tile([P, QL], F32, tag="ps_pro", name="ps_qp")
                for ht in range(HT):
                    nc.tensor.matmul(
                        ps[:],
                        wq_sb[:, ht, dt * P : (dt + 1) * P],
                        qinT[:, ht, :],
                        start=(ht == 0),
                        stop=(ht == HT - 1),
                    )
                nc.scalar.activation(
                    qpT[:, dt, :], ps[:], AF.Identity, bias=bq_sb[:, dt : dt + 1], scale=1.0
                )

            # WQ[h, q] = sum_d Wk[h, d] * Qp^T[d, q]  (lhsT = Wk^T), evac bf16
            for ht in range(HT):
                ps = ps_pro.tile([P, QL], F32, tag="ps_pro", name="ps_wq")
                for dt in range(HT):
                    nc.tensor.matmul(
                        ps[:],
                        wkT[:, dt, ht * P : (ht + 1) * P],
                        qpT[:, dt, :],
                        start=(dt == 0),
                        stop=(dt == HT - 1),
                    )
                nc.vector.tensor_copy(wqq[:, ht, :], ps[:])

        # ---------- main loop over s-chunks ----------
        xpool = ctx.enter_context(tc.tile_pool(name="xb", bufs=4))
        xtpool = ctx.enter_context(tc.tile_pool(name="xt", bufs=3))
        epool = ctx.enter_context(tc.tile_pool(name="eb", bufs=2))

        s0 = 0
        for c, sc in enumerate(chunk_sizes):
            st_n = sc // P
            # X chunk, bf16 (SWDGE cast), with a ones-column at h=H
            xb = xpool.tile([P, ST, H + 1], BF16, tag="xb", name="xb")
            nc.gpsimd.dma_start(
                xb[:, :st_n, :H],
                hs[s0 : s0 + sc, :].rearrange("(t p) h -> p t h", p=P),
            )
            nc.gpsimd.memset(xb[:, :st_n, H : H + 1], 1.0)

            # bounce to DRAM (row-major [s, h]) and transpose-load X^T
            xs = dram.tile([SC, H], BF16, tag="xs", name="xs")
            nc.sync.dma_start(
                xs[:sc, :].rearrange("(t p) h -> p t h", p=P), xb[:, :st_n, :H]
            )
            xt = xtpool.tile([P, HT, SC], BF16, tag="xt", name="xt")
            for ht in range(HT):
                nc.sync.dma_start_transpose(
                    xt[:, ht, :sc], xs[:sc, ht * P : (ht + 1) * P]
                )

            if c == 1:
                # Wv / bv arrive mid-stream; only needed in the epilogue
                nc.sync.dma_start(wvF[:], wv.rearrange("(t p) d -> p t d", p=P))
                nc.sync.dma_start(bvF[:], bv.rearrange("(o d) -> o d", o=1))

            # scores^T tiles + exp
            eb = epool.tile([P, ST, QL], BF16, tag="eb", name="eb")
            for st in range(st_n):
                ps = ps_s.tile([P, QL], F32, tag="ps_s")
                for ht in range(HT):
                    nc.tensor.matmul(
                        ps[:],
                        xt[:, ht, st * P : (st + 1) * P],
                        wqq[:, ht, :],
                        start=(ht == 0),
                        stop=(ht == HT - 1),
                    )
                sg = s0 // P + st
                nc.scalar.activation(
                    eb[:, st, :], ps[:], AF.Exp, bias=mb[:, sg : sg + 1], scale=SCALE
                )

            # T' += E^T @ [X | 1]
            for qt in range(QT):
                for h0, hn in ((0, 512), (512, H + 1 - 512)):
                    ps = ps_t.tile([P, 512], F32, tag="ps_t")
                    for st in range(st_n):
                        nc.tensor.matmul(
                            ps[:, :hn],
                            eb[:, st, qt * P : (qt + 1) * P],
                            xb[:, st, h0 : h0 + hn],
                            start=(st == 0),
                            stop=(st == st_n - 1),
                        )
                    if c == 0:
                        nc.vector.tensor_copy(T_sb[:, qt, h0 : h0 + hn], ps[:, :hn])
                    else:
                        nc.vector.tensor_add(
                            T_sb[:, qt, h0 : h0 + hn],
                            T_sb[:, qt, h0 : h0 + hn],
                            ps[:, :hn],
                        )
            s0 += sc

        # ---------- epilogue ----------
        rs = const.tile([P, QT], F32)
        nc.vector.reciprocal(rs[:], T_sb[:, :, H])

        # T'^T via PE transposes (fp32)
        tT = wpool.tile([P, HT, QL], BF16)
        for ht in range(HT):
            psq = ps_s.tile([P, QL], F32, tag="ps_s", name="ps_tt")
            for qt in range(QT):
                nc.tensor.transpose(
                    psq[:, qt * P : (qt + 1) * P],
                    T_sb[:, qt, ht * P : (ht + 1) * P],
                    ident[:],
                )
            nc.vector.tensor_copy(tT[:, ht, :], psq[:])
        sT = wpool.tile([1, QL], BF16)
        pss = ps_t.tile([1, QL], F32, tag="ps_t", name="ps_st")
        for qt in range(QT):
            nc.tensor.transpose(
                pss[:, qt * P : (qt + 1) * P], T_sb[:, qt, H : H + 1], ident[:]
            )
        nc.vector.tensor_copy(sT[:], pss[:])

        # context[q, d] = (T'^T as lhsT) @ [Wv ; bv], then * 1/sums
        wv_sb = wpool.tile([P, HT, H], BF16)
        nc.scalar.copy(wv_sb[:], wvF[:])
        bv_sb = const.tile([1, H], BF16)
        nc.scalar.copy(bv_sb[:], bvF[:])
        ob = wpool.tile([P, QT, H], F32)
        for qt in range(QT):
            for d0 in (0, 384):
                psf = ps_s.tile([P, QL], F32, tag="ps_s", name="ps_c")
                ps = psf[:, :384]
                for ht in range(HT):
                    nc.tensor.matmul(
                        ps,
                        tT[:, ht, qt * P : (qt + 1) * P],
                        wv_sb[:, ht, d0 : d0 + 384],
                        start=(ht == 0),
                        stop=False,
                    )
                nc.tensor.matmul(
                    ps,
                    sT[:, qt * P : (qt + 1) * P],
                    bv_sb[:, d0 : d0 + 384],
                    start=False,
                    stop=True,
                )
                nc.vector.tensor_scalar_mul(
                    ob[:, qt, d0 : d0 + 384], ps, rs[:, qt : qt + 1]
                )
        nc.sync.dma_start(out.rearrange("(t p) d -> p t d", p=P), ob[:])


def _build():
    nc = bacc.Bacc(
        "TRN2",
        target_bir_lowering=False,
        debug=False,
        enable_asserts=False,
        num_devices=NCORES,
    )
    hs = nc.dram_tensor("hidden_states", [S, H], F32, kind="ExternalInput").ap()
    qin = nc.dram_tensor("queries", [QL, H], F32, kind="ExternalInput").ap()
    msk = nc.dram_tensor("attention_mask", [S], F32, kind="ExternalInput").ap()
    wq = nc.dram_tensor("Wq", [H, H], F32, kind="ExternalInput").ap()
    wk = nc.dram_tensor("Wk", [H, H], F32, kind="ExternalInput").ap()
    wv = nc.dram_tensor("Wv", [H, H], F32, kind="ExternalInput").ap()
    bq = nc.dram_tensor("bq", [H], F32, kind="ExternalInput").ap()
    bv = nc.dram_tensor("bv", [H], F32, kind="ExternalInput").ap()
    out = nc.dram_tensor("out", [QL, H], F32, kind="ExternalOutput").ap()

    with tile.TileContext(nc) as tc:
        _body(nc, tc, hs, qin, msk, wq, wk, wv, bq, bv, out)
    nc.compile()
    return nc


def kernel(**inputs):
    global _cached_nc, LAST_RUN
    if _cached_nc is None:
        _cached_nc = _build()
    nc = _cached_nc

    hs = np.ascontiguousarray(np.asarray(inputs["hidden_states"], dtype=np.float32))
    qs = np.ascontiguousarray(np.asarray(inputs["queries"], dtype=np.float32))
    mk = np.ascontiguousarray(np.asarray(inputs["attention_mask"], dtype=np.float32))
    wq = np.ascontiguousarray(np.asarray(inputs["Wq"], dtype=np.float32))
    wk = np.ascontiguousarray(np.asarray(inputs["Wk"], dtype=np.float32))
    wv = np.ascontiguousarray(np.asarray(inputs["Wv"], dtype=np.float32))
    bq = np.ascontiguousarray(np.asarray(inputs["bq"], dtype=np.float32))
    bv = np.ascontiguousarray(np.asarray(inputs["bv"], dtype=np.float32))

    in_maps = []
    for core in range(NCORES):
        b, qh = divmod(core, 2)
        in_maps.append(
            {
                "hidden_states": hs[b],
                "queries": np.ascontiguousarray(qs[b, qh * QL : (qh + 1) * QL]),
                "attention_mask": mk[b],
                "Wq": wq,
                "Wk": wk,
                "Wv": wv,
                "bq": bq,
                "bv": bv,
            }
        )

    LAST_RUN = bass_utils.run_bass_kernel_spmd(nc, in_maps, core_ids=list(range(NCORES)))
    outf = np.empty((B, QF, H), dtype=np.float32)
    for core in range(NCORES):
        b, qh = divmod(core, 2)
        outf[b, qh * QL : (qh + 1) * QL] = LAST_RUN.results[core]["out"]
    return outf
